# revision 40
# baseline (speedup 1.0000x reference)
"""Causal self-attention (L=4096, D=1024, 16 heads) on 8 TRN2 NeuronCores.

Sharding: tensor-parallel over heads — each core owns 2 heads (128 head-dims).
Per core:
  QT/KT = W @ x.T (+bias)          [128, L]   (head-dims on partitions)
  V     = x @ Wv.T (+bias)         [L, 128]   (tokens on partitions)
  S.T   = K @ Q.T  (per head)      [k, q] blocks, causal-skipped
  E     = exp(S.T/8) * mask        (no max-subtraction: |logits| < ~3.1)
  O.T   = [V|1].T @ E              -> unnormalized head outputs + col-sums
  O.T  /= sums  (PE broadcast + DVE reciprocal)
  partial = O @ Wo_slice.T         [L, D]
Host: out = sum_c(partial_c) + b_out.

All matmuls bf16 with fp32 PSUM accumulation. The exp is split across two
engines: ACT runs true exp; the DVE runs a Schraudolph bf16 approximation
(int16 affine of the raw scores whose bits ARE bf16(exp)), which also fuses
the causal mask via a per-element bias tile that drops masked entries into
bf16-denormal territory. Diagonal blocks restrict scores/exp/ppv to the
causally reachable column range. ppv is software-pipelined one iteration
behind exp so the in-order PE queue never waits on a just-launched exp, and
projection work runs two chunks ahead. Measured rel err ~1.1e-2.
"""

import numpy as np
import ml_dtypes

import concourse.bass as bass
import concourse.mybir as mybir
import concourse.tile as tile
from concourse import bacc
from concourse.bass import ts
from concourse.bass_utils import run_bass_kernel_spmd

L, D = 4096, 1024
P = 128
NCORES = 8
HDC = 128          # head-dims per core (2 heads x 64)
KO = D // P        # 8 contraction chunks of the model dim
NJ = L // 512      # 8 q-chunks of 512
NK = L // P        # 32 k-chunks of 128
BF16 = mybir.dt.bfloat16
F32 = mybir.dt.float32
F8E4 = mybir.dt.float8e4
I16 = mybir.dt.int16
I8 = mybir.dt.int8
EXP = mybir.ActivationFunctionType.Exp
IDENT = mybir.ActivationFunctionType.Identity
MUL = mybir.AluOpType.mult
ADD = mybir.AluOpType.add
DR = mybir.MatmulPerfMode.DoubleRow

# Schraudolph bf16 exp: bits_i16 = SCA*s + SCB approximates bf16(exp(s/8)).
# SCA = 128/ln(2)/8; SCB = 127*128 - 5.59 (mantissa-linearization shift).
# Masked positions add -15600 so bits land in [0, ~1300) -> bf16 denormal ~ 0.
SCA = 23.083120654223414
SCB = 16250.41
SCMASK = -15600.0
# Schraudolph fp8-e4m3 exp for the DoubleRow path: bits_i8 = SCA8*s + SCB8
# approximates the e4m3 bit pattern of exp(s/8) (3-bit mantissa).
SCA8 = 8.0 / (8.0 * float(np.log(2.0)))
SCB8 = 56.0 - 0.45
# non-diag exp engine mix: i % EXPMOD in ACT_SLOTS -> ACT, else DVE schr8
EXPMOD = 10
NACT = 7
# outproj tiles handled per chunk (prefix sums give each chunk's start tile)
OP_SHARE = [0, 0, 4, 4, 4, 4, 4, 8]
OP_START = [0, 0, 0, 4, 8, 12, 16, 20]


def _build():
    nc = bacc.Bacc("TRN2", target_bir_lowering=False)

    xt_d = nc.dram_tensor("xt", [P, NJ, KO, 512], BF16, kind="ExternalInput")
    wq_d = nc.dram_tensor("wq", [P, KO, HDC], BF16, kind="ExternalInput")
    wk_d = nc.dram_tensor("wk", [P, KO, HDC], BF16, kind="ExternalInput")
    wv_d = nc.dram_tensor("wv", [P, KO, HDC], BF16, kind="ExternalInput")
    wo_d = nc.dram_tensor("wo", [HDC, D], BF16, kind="ExternalInput")
    bqk_d = nc.dram_tensor("bqk", [HDC, 2], F32, kind="ExternalInput")
    # [bv | ones128 | ones64] packed into one row-tensor
    cst_d = nc.dram_tensor("cst", [1, 320], BF16, kind="ExternalInput")
    out_d = nc.dram_tensor("out", [L, D], BF16, kind="ExternalOutput")

    # Schraudolph bias-with-mask for the 4 diagonal-block offsets, per head:
    # bmask[p, m, h, c] = SCB if c >= p + 128*m else SCB + SCMASK (pushes the
    # int16 bits into bf16-denormal land -> e ~= 0).
    qi = np.arange(512)
    half = (
        qi[None, None, :] >= (np.arange(P)[:, None, None] + 128 * np.arange(4)[None, :, None])
    )
    mask_np = np.stack([half, half], axis=2)          # [P, 4, 2, 512]
    bmask_np = np.where(mask_np, np.float32(SCB), np.float32(SCB + SCMASK))
    mask_d = nc.inline_tensor(np.ascontiguousarray(bmask_np.astype(np.float32)), name="maskc")

    with tile.TileContext(nc) as tc:
        with (
            tc.tile_pool(name="const", bufs=1) as cp,
            tc.tile_pool(name="work", bufs=4) as wp,
            tc.tile_pool(name="psum", bufs=1, space="PSUM") as pp,
        ):
            # ---- first-needed DMAs first: wq/wk + token-chunk-0/1 of x gate
            # the first projections; everything else is deferred behind them.
            wq = cp.tile([P, KO, HDC], BF16, name="wq_s", tag="wq_s")
            wk = cp.tile([P, KO, HDC], BF16, name="wk_s", tag="wk_s")
            wv = cp.tile([P, KO, HDC], BF16, name="wv_s", tag="wv_s")
            xta = cp.tile([P, NJ, KO, 512], BF16, name="xta", tag="xta")
            maskt = cp.tile([P, 4, 2, 512], F32, name="mask_s", tag="mask_s")
            wo = cp.tile([P, D], BF16, name="wo_s", tag="wo_s")
            bqk = cp.tile([P, 2], F32, name="bqk_s", tag="bqk_s")
            cst = cp.tile([1, 320], BF16, name="cst_s", tag="cst_s")
            bv = cst[0:1, 0:128]
            ones1 = cst[0:1, 128:256]
            ones64 = cst[0:1, 256:320]
            bq = bqk[:, 0:1]
            bk = bqk[:, 1:2]
            # first-needed pieces lead each HW DMA queue so the first proj
            # matmuls (fp8 wq x xt8[:,0]) can start as early as possible; the
            # mask (for j=0 diag exp) streams per-m right behind them.
            nc.scalar.dma_start(wq[:, 0:1], wq_d[:, 0:1])
            nc.sync.dma_start(xta[:, 0, 0:1], xt_d[:, 0, 0:1])
            nc.scalar.dma_start(wq[:, 1:8], wq_d[:, 1:8])
            nc.sync.dma_start(xta[:, 0, 1:5], xt_d[:, 0, 1:5])
            nc.scalar.dma_start(wk[:], wk_d[:])
            nc.sync.dma_start(xta[:, 0, 5:8], xt_d[:, 0, 5:8])
            nc.scalar.dma_start(bqk[:], bqk_d[:])
            nc.sync.dma_start(maskt[:, 0:2], mask_d[:, 0:2])
            nc.scalar.dma_start(wv[:], wv_d[:])
            nc.scalar.dma_start(cst[:], cst_d[:])
            nc.sync.dma_start(maskt[:, 2:4], mask_d[:, 2:4])
            nc.scalar.dma_start(xta[:, 1], xt_d[:, 1])
            nc.sync.dma_start(wo[:], wo_d[:])
            for jcol in range(2, NJ):
                nc.sync.dma_start(xta[:, jcol], xt_d[:, jcol])

            qt = [cp.tile([P, 512], BF16, name=f"qt{j}", tag=f"qt{j}") for j in range(NJ)]
            kt = [cp.tile([P, 512], BF16, name=f"kt{j}", tag=f"kt{j}") for j in range(NJ)]
            ot = [cp.tile([P, 512], BF16, name=f"ot{j}", tag=f"ot{j}") for j in range(NJ)]
            v01 = [cp.tile([P, 2, 65], BF16, name=f"v01_{i}", tag=f"v01_{i}") for i in range(NK)]
            bvr = cp.tile([P, 2, 64], BF16, name="bvr_s", tag="bvr_s")
            # fp8 [V|1] pair tiles for DoubleRow ppv: [slot(2), head(2), 80]
            # (pair p packs k-tiles 2p/2p+1; tiles 28..31 are diag-only)
            v8 = [
                cp.tile([P, 2, 2, 80], F8E4, name=f"v8_{p}", tag=f"v8_{p}")
                for p in range(14)
            ]

            ppv = {}  # j -> (ppv0, ppv1) accumulation psums kept until epilogue

            def proj_qk(g, which):
                """QT or KT projection for token chunk g (bias add: q on ACT,
                k on DVE, for engine balance)."""
                w, b, dst, nm = (wq, bq, qt, "q") if which == "q" else (wk, bk, kt, "k")
                ps = pp.tile([P, 512], F32, name=f"ps{nm}{g}", tag="mx", bufs=2)
                for k in range(KO):
                    nc.tensor.matmul(
                        ps[:], w[:, k, :], xta[:, g, k, :],
                        start=(k == 0), stop=(k == KO - 1),
                    )
                if which == "q":
                    nc.scalar.activation(dst[g][:], ps[:], IDENT, bias=b)
                else:
                    nc.vector.tensor_scalar_add(dst[g][:], ps[:], b)

            def proj_v(t):
                """V projection for token tile t (both heads + bias + ones col)."""
                psv = pp.tile([P, 2, 64], F32, name=f"psv{t}", tag="mx", bufs=2)
                for k in range(KO):
                    nc.tensor.matmul(
                        psv[:], xta[:, t // 4, k, ts(t % 4, P)], wv[:, k, :],
                        start=(k == 0), stop=(k == KO - 1),
                    )
                nc.vector.tensor_tensor(
                    v01[t][:, :, 0:64], psv[:], bvr[:], mybir.AluOpType.add
                )
                if t < 28:
                    # fp8 copy for the DoubleRow path (gpsimd is otherwise idle)
                    nc.gpsimd.tensor_copy(
                        v8[t // 2][:, t % 2, :, 0:64], v01[t][:, :, 0:64]
                    )

            def normalize(j):
                """Normalize chunk j's head outputs into ot[j]."""
                ppv0, ppv1 = ppv.pop(j)
                s0 = wp.tile([1, 512], BF16, name=f"s0_{j}", tag="s0", bufs=4)
                s1 = wp.tile([1, 512], BF16, name=f"s1_{j}", tag="s1", bufs=4)
                nc.scalar.copy(s0[:], ppv0[64:65, :])
                nc.vector.tensor_copy(s1[:], ppv1[64:65, :])
                pb = pp.tile([P, 512], F32, name=f"pb_{j}", tag="mx", bufs=2)
                nc.tensor.matmul(pb[0:64, :], ones64, s0[:], start=True, stop=True,
                                 tile_position=(0, 0))
                nc.tensor.matmul(pb[64:128, :], ones64, s1[:], start=True, stop=True,
                                 tile_position=(0, 64))
                rc = wp.tile([P, 512], F32, name=f"rc_{j}", tag="rc", bufs=4)
                nc.vector.reciprocal_approx_fast(rc[:], pb[:])
                nc.vector.tensor_mul(ot[j][0:64, :], ppv0[0:64, :], rc[0:64, :])
                nc.vector.tensor_mul(ot[j][64:128, :], ppv1[0:64, :], rc[64:128, :])

            def outproj(j, t):
                ob = wp.tile([P, D], BF16, name=f"ob_{t}", tag="ob", bufs=8)
                for n in range(2):
                    po = pp.tile([P, 512], F32, name=f"po_{t}_{n}", tag="mx", bufs=2)
                    nc.tensor.matmul(
                        po[:], ot[j][:, ts(t - 4 * j, P)], wo[:, ts(n, 512)],
                        start=True, stop=True,
                    )
                    # each half DMAs right after its own copy, on its own queue
                    if n == 1:
                        nc.scalar.copy(ob[:, ts(n, 512)], po[:])
                        nc.scalar.dma_start(out_d[ts(t, P), ts(n, 512)], ob[:, ts(n, 512)])
                    else:
                        nc.vector.tensor_copy(ob[:, ts(n, 512)], po[:])
                        nc.sync.dma_start(out_d[ts(t, P), ts(n, 512)], ob[:, ts(n, 512)])

            jf = NJ - 1

            def final_slice(sl):
                """Normalize + out-project one 128-token slice of the last
                chunk (its ppv columns are final after iteration 4*jf+sl)."""
                fpv0, fpv1 = ppv[jf]
                c = ts(sl, P)
                s0 = wp.tile([1, P], BF16, name=f"s0f_{sl}", tag="s0f", bufs=4)
                s1 = wp.tile([1, P], BF16, name=f"s1f_{sl}", tag="s1f", bufs=4)
                nc.scalar.copy(s0[:], fpv0[64:65, c])
                nc.vector.tensor_copy(s1[:], fpv1[64:65, c])
                pbf = pp.tile([P, P], F32, name=f"pbf_{sl}", tag="mx", bufs=2)
                nc.tensor.matmul(pbf[0:64, :], ones64, s0[:], start=True, stop=True,
                                 tile_position=(0, 0))
                nc.tensor.matmul(pbf[64:128, :], ones64, s1[:], start=True, stop=True,
                                 tile_position=(0, 64))
                rcf = wp.tile([P, P], F32, name=f"rcf_{sl}", tag="rcf", bufs=4)
                nc.vector.reciprocal_approx_fast(rcf[:], pbf[:])
                nc.vector.tensor_mul(ot[jf][0:64, c], fpv0[0:64, c], rcf[0:64, :])
                nc.vector.tensor_mul(ot[jf][64:128, c], fpv1[0:64, c], rcf[64:128, :])
                outproj(jf, 4 * jf + sl)

            # HAM warm-up: ~3.5us of throwaway matmuls on memset scratch while
            # the first DMAs are in flight, so the real matmuls start at the
            # warm 2.4 GHz clock instead of the cold 1.2 GHz default.
            wsc = cp.tile([P, 512], BF16, name="wsc", tag="wsc")
            nc.gpsimd.memset(wsc[:], 0.0)
            wps = pp.tile([P, 512], F32, name="wps", tag="mx", bufs=2)
            for _ in range(9):
                nc.tensor.matmul(wps[:], wsc[:, 0:128], wsc[:], start=True, stop=True)

            # projections for chunk 0 up front
            proj_qk(0, "q")
            proj_qk(0, "k")
            pbv = pp.tile([P, 2, 64], F32, name="pbv", tag="mx", bufs=2)
            nc.tensor.matmul(pbv[:], ones1, bv, start=True, stop=True)
            nc.vector.tensor_copy(bvr[:], pbv[:])
            for i in range(NK):
                nc.gpsimd.memset(v01[i][:, :, 64:65], 1.0)
            for p in range(14):
                nc.gpsimd.memset(v8[p][:, :, :, 64:65], 1.0)

            nonstate = {"nexp": 0}
            for g in range(NJ):
                j = g
                nkj = 4 * (j + 1)
                ppv0 = pp.tile([65, 512], F32, name=f"ppv0_{j}", tag="ppv0", bufs=1)
                ppv1 = pp.tile([65, 512], F32, name=f"ppv1_{j}", tag="ppv1", bufs=1)
                ppv[j] = (ppv0, ppv1)

                # work units spread across this i-loop: projections for chunk
                # g+1 and the out-projection of the already-normalized chunk g-1
                units = []
                if g + 1 < NJ:
                    units.append(lambda g=g: proj_qk(g + 1, "q"))
                if g >= 1:
                    # this chunk's own V tiles: consumed at i >= 4g (the
                    # diagonal); schedule them early so the diag ppv LDWEIGHTS
                    # never waits on the DVE bias-add that produces v01
                    for t in range(4 * g, 4 * g + 2):
                        units.append(lambda t=t: proj_v(t))
                if g + 1 < NJ:
                    units.append(lambda g=g: proj_qk(g + 1, "k"))
                if g >= 1:
                    for t in range(4 * g + 2, 4 * g + 4):
                        units.append(lambda t=t: proj_v(t))
                # out-projection tiles spread in proportion to chunk length so
                # the 2-buffer mx psum pool never has 3 users in flight: chunk
                # g handles OP_SHARE[g] tiles in order (tile t of chunk t//4
                # becomes eligible 2 chunks later; chunk 6's tiles must go in
                # chunk 7 right after normalize(6)).
                for t in range(OP_START[g], OP_START[g] + OP_SHARE[g]):
                    units.append(lambda t=t: outproj(t // 4, t))
                nu = len(units)
                slots = {}
                for u in range(nu):
                    slots.setdefault(min(nkj - 1, 1 + (u * nkj) // (nu + 1)), []).append(units[u])

                # software-pipelined with lag 2+: exp gets >= two block-walls
                # before the in-order PE queue consumes it. Non-diag blocks go
                # through fp8 e + fp8 [V|1] DoubleRow matmuls (two k-tiles per
                # MM); diagonal blocks keep the masked bf16 Schraudolph path.
                # Scores psum bufs=2 suffices: exp(i) completes before
                # scores(i+2) needs its buffer.
                et = {}
                e8s = {}
                # diag blocks consume at lag dlag (3 normally, 1 on the last
                # chunk: the PE has nothing else left, so chase the epilogue);
                # fp8 pairs at ii = 2p+4 so they never wait on a FIFO'd exp.
                # PSUM accumulation is additive, so the j=NJ-1 interleaving of
                # diag blocks before the last pairs is safe (start was pair 0).
                dlag = 1 if j == NJ - 1 else 3
                for ii in range(nkj + dlag):
                    if ii < nkj:
                        i = ii
                        ps = pp.tile([P, 2, 512], F32, name=f"ps_{j}_{i}", tag="s", bufs=2)
                        m = i - 4 * j
                        c0 = 128 * m if m >= 1 else 0
                        nc.tensor.matmul(
                            ps[:, 0, c0:512], kt[i // 4][0:64, ts(i % 4, P)],
                            qt[j][0:64, c0:512], start=True, stop=True,
                        )
                        nc.tensor.matmul(
                            ps[:, 1, c0:512], kt[i // 4][64:128, ts(i % 4, P)],
                            qt[j][64:128, c0:512], start=True, stop=True,
                        )
                        if m >= 0:
                            e = wp.tile([P, 2, 512], BF16, name=f"e_{j}_{i}", tag="e", bufs=6)
                            et[i] = (e, c0)
                            # Schraudolph exp with fused causal mask on DVE
                            nc.vector.scalar_tensor_tensor(
                                e[:, :, c0:512].bitcast(I16), ps[:, :, c0:512],
                                SCA, maskt[:, m, :, c0:512], op0=MUL, op1=ADD,
                            )
                        else:
                            p8 = i // 2
                            if i % 2 == 0:
                                e8s[p8] = wp.tile(
                                    [P, 2, 2, 512], F8E4,
                                    name=f"e8_{j}_{p8}", tag="e8", bufs=4,
                                )
                            e8t = e8s[p8]
                            self_i = nonstate["nexp"]
                            nonstate["nexp"] += 1
                            if self_i % EXPMOD < NACT:
                                nc.scalar.activation(
                                    e8t[:, i % 2, :, :], ps[:], EXP, scale=0.125
                                )
                            else:
                                nc.vector.tensor_scalar(
                                    e8t[:, i % 2, :, :].bitcast(I8), ps[:],
                                    SCA8, SCB8, op0=MUL, op1=ADD,
                                )
                        if j == 0:
                            proj_v(i)
                        if i == 0 and j > 0:
                            normalize(j - 1)
                    if ii >= 4 and ii % 2 == 0 and (ii - 4) // 2 < 2 * j:
                        # DoubleRow pair (2p, 2p+1): fp8 e x fp8 [V|1]
                        p8 = (ii - 4) // 2
                        e8t = e8s.pop(p8)
                        nc.tensor.matmul(
                            ppv0[:], v8[p8][:, :, 0, 0:65], e8t[:, :, 0, :],
                            perf_mode=DR, start=(p8 == 0), stop=False,
                        )
                        nc.tensor.matmul(
                            ppv1[:], v8[p8][:, :, 1, 0:65], e8t[:, :, 1, :],
                            perf_mode=DR, start=(p8 == 0), stop=False,
                        )
                    if ii >= dlag and ii - dlag >= 4 * j:
                        i = ii - dlag
                        e, c0 = et.pop(i)
                        nc.tensor.matmul(
                            ppv0[:, c0:512], v01[i][:, 0, :], e[:, 0, c0:512],
                            start=(i == 0 and j == 0), stop=(i == nkj - 1),
                        )
                        nc.tensor.matmul(
                            ppv1[:, c0:512], v01[i][:, 1, :], e[:, 1, c0:512],
                            start=(i == 0 and j == 0), stop=(i == nkj - 1),
                        )
                        # last chunk: with column restriction, ppv columns
                        # [128*sl, 128*sl+128) are final once iteration
                        # i = 4*j + sl has accumulated -> stream the epilogue
                        # slice-by-slice instead of serializing it at the end
                        if j == NJ - 1 and 0 <= i - 4 * j < 4:
                            final_slice(i - 4 * j)
                    for fn in slots.get(ii, []):
                        fn()

    nc.compile()
    return nc


def _pack_w(w_slice, dt=ml_dtypes.bfloat16):
    """[HDC, D] weight slice -> transposed, chunked [P, KO, HDC]."""
    wt = np.ascontiguousarray(w_slice.T)          # [D, HDC]
    return np.ascontiguousarray(
        wt.reshape(KO, P, HDC).transpose(1, 0, 2)
    ).astype(dt)


def _make_in_maps(x, W_qkv, b_qkv, W_out, b_out):
    bf = ml_dtypes.bfloat16
    x = np.asarray(x, np.float32)
    W_qkv = np.asarray(W_qkv, np.float32)
    b_qkv = np.asarray(b_qkv, np.float32)
    W_out = np.asarray(W_out, np.float32)
    xt = np.ascontiguousarray(
        x.T.reshape(KO, P, NJ, 512).transpose(1, 2, 0, 3)
    ).astype(bf)
    in_maps = []
    for c in range(NCORES):
        r = slice(HDC * c, HDC * (c + 1))
        in_maps.append({
            "xt": xt,
            "wq": _pack_w(W_qkv[0 * D:1 * D][r]),
            "wk": _pack_w(W_qkv[1 * D:2 * D][r]),
            "wv": _pack_w(W_qkv[2 * D:3 * D][r]),
            "wo": np.ascontiguousarray(W_out[:, r].T).astype(bf),
            "bqk": np.ascontiguousarray(
                np.stack(
                    [b_qkv[0 * D:1 * D][r], b_qkv[1 * D:2 * D][r]], axis=1
                )
            ).astype(np.float32),
            "cst": np.ascontiguousarray(
                np.concatenate(
                    [b_qkv[2 * D:3 * D][r], np.ones(192, np.float32)]
                )[None, :]
            ).astype(bf),
        })
    return in_maps


_NC_CACHE = {}


def kernel(x, W_qkv, b_qkv, W_out, b_out):
    if "nc" not in _NC_CACHE:
        _NC_CACHE["nc"] = _build()
    nc = _NC_CACHE["nc"]
    in_maps = _make_in_maps(x, W_qkv, b_qkv, W_out, b_out)
    res = run_bass_kernel_spmd(nc, in_maps, core_ids=list(range(NCORES)))
    out = np.zeros((L, D), np.float32)
    for c in range(NCORES):
        out += res.results[c]["out"].astype(np.float32)
    out += np.asarray(b_out, np.float32)[None, :]
    return out



# revision 42
# speedup vs baseline: 1.0144x; 1.0144x over previous
"""Causal self-attention (L=4096, D=1024, 16 heads) on 8 TRN2 NeuronCores.

Sharding: tensor-parallel over heads — each core owns 2 heads (128 head-dims).
Per core:
  QT/KT = W @ x.T (+bias)          [128, L]   (head-dims on partitions)
  V     = x @ Wv.T (+bias)         [L, 128]   (tokens on partitions)
  S.T   = K @ Q.T  (per head)      [k, q] blocks, causal-skipped
  E     = exp(S.T/8) * mask        (no max-subtraction: |logits| < ~3.1)
  O.T   = [V|1].T @ E              -> unnormalized head outputs + col-sums
  O.T  /= sums  (PE broadcast + DVE reciprocal)
  partial = O @ Wo_slice.T         [L, D]
Host: out = sum_c(partial_c) + b_out.

All matmuls bf16 with fp32 PSUM accumulation. The exp is split across two
engines: ACT runs true exp; the DVE runs a Schraudolph bf16 approximation
(int16 affine of the raw scores whose bits ARE bf16(exp)), which also fuses
the causal mask via a per-element bias tile that drops masked entries into
bf16-denormal territory. Diagonal blocks restrict scores/exp/ppv to the
causally reachable column range. ppv is software-pipelined one iteration
behind exp so the in-order PE queue never waits on a just-launched exp, and
projection work runs two chunks ahead. Measured rel err ~1.1e-2.
"""

import numpy as np
import ml_dtypes

import concourse.bass as bass
import concourse.mybir as mybir
import concourse.tile as tile
from concourse import bacc
from concourse.bass import ts
from concourse.bass_utils import run_bass_kernel_spmd

L, D = 4096, 1024
P = 128
NCORES = 8
HDC = 128          # head-dims per core (2 heads x 64)
KO = D // P        # 8 contraction chunks of the model dim
NJ = L // 512      # 8 q-chunks of 512
NK = L // P        # 32 k-chunks of 128
BF16 = mybir.dt.bfloat16
F32 = mybir.dt.float32
F8E4 = mybir.dt.float8e4
I16 = mybir.dt.int16
I8 = mybir.dt.int8
EXP = mybir.ActivationFunctionType.Exp
IDENT = mybir.ActivationFunctionType.Identity
MUL = mybir.AluOpType.mult
ADD = mybir.AluOpType.add
DR = mybir.MatmulPerfMode.DoubleRow

# Schraudolph bf16 exp: bits_i16 = SCA*s + SCB approximates bf16(exp(s/8)).
# SCA = 128/ln(2)/8; SCB = 127*128 - 5.59 (mantissa-linearization shift).
# Masked positions add -15600 so bits land in [0, ~1300) -> bf16 denormal ~ 0.
SCA = 23.083120654223414
SCB = 16250.41
SCMASK = -15600.0
# Schraudolph fp8-e4m3 exp for the DoubleRow path: bits_i8 = SCA8*s + SCB8
# approximates the e4m3 bit pattern of exp(s/8) (3-bit mantissa).
SCA8 = 8.0 / (8.0 * float(np.log(2.0)))
SCB8 = 56.0 - 0.45
# non-diag exp engine mix: i % EXPMOD in ACT_SLOTS -> ACT, else DVE schr8
EXPMOD = 10
NACT = 7
# outproj tiles handled per chunk (prefix sums give each chunk's start tile)
OP_SHARE = [0, 0, 4, 4, 4, 4, 4, 8]
OP_START = [0, 0, 0, 4, 8, 12, 16, 20]


def _build():
    nc = bacc.Bacc("TRN2", target_bir_lowering=False)

    xt_d = nc.dram_tensor("xt", [P, NJ, KO, 512], BF16, kind="ExternalInput")
    wq_d = nc.dram_tensor("wq", [P, KO, HDC], BF16, kind="ExternalInput")
    wk_d = nc.dram_tensor("wk", [P, KO, HDC], BF16, kind="ExternalInput")
    wv_d = nc.dram_tensor("wv", [P, KO, HDC], BF16, kind="ExternalInput")
    wo_d = nc.dram_tensor("wo", [HDC, D], BF16, kind="ExternalInput")
    bqk_d = nc.dram_tensor("bqk", [HDC, 2], F32, kind="ExternalInput")
    # [bv | ones128 | ones64] packed into one row-tensor
    cst_d = nc.dram_tensor("cst", [1, 320], BF16, kind="ExternalInput")
    out_d = nc.dram_tensor("out", [L, D], BF16, kind="ExternalOutput")

    # Schraudolph bias-with-mask for the 4 diagonal-block offsets, per head:
    # bmask[p, m, h, c] = SCB if c >= p + 128*m else SCB + SCMASK (pushes the
    # int16 bits into bf16-denormal land -> e ~= 0).
    qi = np.arange(512)
    half = (
        qi[None, None, :] >= (np.arange(P)[:, None, None] + 128 * np.arange(4)[None, :, None])
    )
    mask_np = np.stack([half, half], axis=2)          # [P, 4, 2, 512]
    bmask_np = np.where(mask_np, np.float32(SCB), np.float32(SCB + SCMASK))
    mask_d = nc.inline_tensor(np.ascontiguousarray(bmask_np.astype(np.float32)), name="maskc")

    with tile.TileContext(nc) as tc:
        with (
            tc.tile_pool(name="const", bufs=1) as cp,
            tc.tile_pool(name="work", bufs=4) as wp,
            tc.tile_pool(name="psum", bufs=1, space="PSUM") as pp,
        ):
            # ---- first-needed DMAs first: wq/wk + token-chunk-0/1 of x gate
            # the first projections; everything else is deferred behind them.
            wq = cp.tile([P, KO, HDC], BF16, name="wq_s", tag="wq_s")
            wk = cp.tile([P, KO, HDC], BF16, name="wk_s", tag="wk_s")
            wv = cp.tile([P, KO, HDC], BF16, name="wv_s", tag="wv_s")
            xta = cp.tile([P, NJ, KO, 512], BF16, name="xta", tag="xta")
            maskt = cp.tile([P, 4, 2, 512], F32, name="mask_s", tag="mask_s")
            wo = cp.tile([P, D], BF16, name="wo_s", tag="wo_s")
            bqk = cp.tile([P, 2], F32, name="bqk_s", tag="bqk_s")
            cst = cp.tile([1, 320], BF16, name="cst_s", tag="cst_s")
            bv = cst[0:1, 0:128]
            ones1 = cst[0:1, 128:256]
            ones64 = cst[0:1, 256:320]
            bq = bqk[:, 0:1]
            bk = bqk[:, 1:2]
            # first-needed pieces lead each HW DMA queue so the first proj
            # matmuls (fp8 wq x xt8[:,0]) can start as early as possible; the
            # mask (for j=0 diag exp) streams per-m right behind them.
            nc.scalar.dma_start(wq[:, 0:1], wq_d[:, 0:1])
            nc.sync.dma_start(xta[:, 0, 0:1], xt_d[:, 0, 0:1])
            nc.scalar.dma_start(wq[:, 1:8], wq_d[:, 1:8])
            nc.sync.dma_start(xta[:, 0, 1:5], xt_d[:, 0, 1:5])
            nc.scalar.dma_start(wk[:], wk_d[:])
            nc.sync.dma_start(xta[:, 0, 5:8], xt_d[:, 0, 5:8])
            nc.scalar.dma_start(bqk[:], bqk_d[:])
            nc.sync.dma_start(maskt[:, 0:2], mask_d[:, 0:2])
            nc.scalar.dma_start(wv[:], wv_d[:])
            nc.scalar.dma_start(cst[:], cst_d[:])
            nc.sync.dma_start(maskt[:, 2:4], mask_d[:, 2:4])
            nc.scalar.dma_start(xta[:, 1], xt_d[:, 1])
            nc.sync.dma_start(wo[:], wo_d[:])
            for jcol in range(2, NJ):
                nc.sync.dma_start(xta[:, jcol], xt_d[:, jcol])

            # HAM warm-up: ~3.5us of throwaway matmuls on memset scratch while
            # the first DMAs are in flight, so the real matmuls start at the
            # warm 2.4 GHz clock instead of the cold 1.2 GHz default. The
            # memset goes first on the (empty) gpsimd queue.
            wsc = cp.tile([P, 512], BF16, name="wsc", tag="wsc")
            nc.gpsimd.memset(wsc[:], 0.0)
            wps = pp.tile([P, 512], F32, name="wps", tag="mx", bufs=2)
            for _ in range(8):
                nc.tensor.matmul(wps[:], wsc[:, 0:128], wsc[:], start=True, stop=True)

            qt = [cp.tile([P, 512], BF16, name=f"qt{j}", tag=f"qt{j}") for j in range(NJ)]
            kt = [cp.tile([P, 512], BF16, name=f"kt{j}", tag=f"kt{j}") for j in range(NJ)]
            ot = [cp.tile([P, 512], BF16, name=f"ot{j}", tag=f"ot{j}") for j in range(NJ)]
            v01 = [cp.tile([P, 2, 65], BF16, name=f"v01_{i}", tag=f"v01_{i}") for i in range(NK)]
            bvr = cp.tile([P, 2, 64], BF16, name="bvr_s", tag="bvr_s")
            # fp8 [V|1] pair tiles for DoubleRow ppv: [slot(2), head(2), 80]
            # (pair p packs k-tiles 2p/2p+1; tiles 28..31 are diag-only)
            v8 = [
                cp.tile([P, 2, 2, 80], F8E4, name=f"v8_{p}", tag=f"v8_{p}")
                for p in range(14)
            ]

            ppv = {}  # j -> (ppv0, ppv1) accumulation psums kept until epilogue

            def proj_qk(g, which):
                """QT or KT projection for token chunk g (bias add: q on ACT,
                k on DVE, for engine balance)."""
                w, b, dst, nm = (wq, bq, qt, "q") if which == "q" else (wk, bk, kt, "k")
                ps = pp.tile([P, 512], F32, name=f"ps{nm}{g}", tag="mx", bufs=2)
                for k in range(KO):
                    nc.tensor.matmul(
                        ps[:], w[:, k, :], xta[:, g, k, :],
                        start=(k == 0), stop=(k == KO - 1),
                    )
                if which == "q":
                    nc.scalar.activation(dst[g][:], ps[:], IDENT, bias=b)
                else:
                    nc.vector.tensor_scalar_add(dst[g][:], ps[:], b)

            def proj_v(t):
                """V projection for token tile t (both heads + bias + ones col)."""
                psv = pp.tile([P, 2, 64], F32, name=f"psv{t}", tag="mx", bufs=2)
                for k in range(KO):
                    nc.tensor.matmul(
                        psv[:], xta[:, t // 4, k, ts(t % 4, P)], wv[:, k, :],
                        start=(k == 0), stop=(k == KO - 1),
                    )
                nc.vector.tensor_tensor(
                    v01[t][:, :, 0:64], psv[:], bvr[:], mybir.AluOpType.add
                )
                if t < 28:
                    # fp8 copy for the DoubleRow path (gpsimd is otherwise idle)
                    nc.gpsimd.tensor_copy(
                        v8[t // 2][:, t % 2, :, 0:64], v01[t][:, :, 0:64]
                    )

            def normalize(j):
                """Normalize chunk j's head outputs into ot[j]."""
                ppv0, ppv1 = ppv.pop(j)
                s0 = wp.tile([1, 512], BF16, name=f"s0_{j}", tag="s0", bufs=4)
                s1 = wp.tile([1, 512], BF16, name=f"s1_{j}", tag="s1", bufs=4)
                nc.scalar.copy(s0[:], ppv0[64:65, :])
                nc.vector.tensor_copy(s1[:], ppv1[64:65, :])
                pb = pp.tile([P, 512], F32, name=f"pb_{j}", tag="mx", bufs=2)
                nc.tensor.matmul(pb[0:64, :], ones64, s0[:], start=True, stop=True,
                                 tile_position=(0, 0))
                nc.tensor.matmul(pb[64:128, :], ones64, s1[:], start=True, stop=True,
                                 tile_position=(0, 64))
                rc = wp.tile([P, 512], F32, name=f"rc_{j}", tag="rc", bufs=4)
                nc.vector.reciprocal_approx_fast(rc[:], pb[:])
                nc.vector.tensor_mul(ot[j][0:64, :], ppv0[0:64, :], rc[0:64, :])
                nc.vector.tensor_mul(ot[j][64:128, :], ppv1[0:64, :], rc[64:128, :])

            def outproj(j, t):
                ob = wp.tile([P, D], BF16, name=f"ob_{t}", tag="ob", bufs=8)
                for n in range(2):
                    po = pp.tile([P, 512], F32, name=f"po_{t}_{n}", tag="mx", bufs=2)
                    nc.tensor.matmul(
                        po[:], ot[j][:, ts(t - 4 * j, P)], wo[:, ts(n, 512)],
                        start=True, stop=True,
                    )
                    # each half DMAs right after its own copy, on its own queue
                    if n == 1:
                        nc.scalar.copy(ob[:, ts(n, 512)], po[:])
                        nc.scalar.dma_start(out_d[ts(t, P), ts(n, 512)], ob[:, ts(n, 512)])
                    else:
                        nc.vector.tensor_copy(ob[:, ts(n, 512)], po[:])
                        nc.sync.dma_start(out_d[ts(t, P), ts(n, 512)], ob[:, ts(n, 512)])

            jf = NJ - 1

            def final_slice(sl):
                """Normalize + out-project one 128-token slice of the last
                chunk (its ppv columns are final after iteration 4*jf+sl)."""
                fpv0, fpv1 = ppv[jf]
                c = ts(sl, P)
                s0 = wp.tile([1, P], BF16, name=f"s0f_{sl}", tag="s0f", bufs=4)
                s1 = wp.tile([1, P], BF16, name=f"s1f_{sl}", tag="s1f", bufs=4)
                nc.scalar.copy(s0[:], fpv0[64:65, c])
                nc.vector.tensor_copy(s1[:], fpv1[64:65, c])
                pbf = pp.tile([P, P], F32, name=f"pbf_{sl}", tag="mx", bufs=2)
                nc.tensor.matmul(pbf[0:64, :], ones64, s0[:], start=True, stop=True,
                                 tile_position=(0, 0))
                nc.tensor.matmul(pbf[64:128, :], ones64, s1[:], start=True, stop=True,
                                 tile_position=(0, 64))
                rcf = wp.tile([P, P], F32, name=f"rcf_{sl}", tag="rcf", bufs=4)
                nc.vector.reciprocal_approx_fast(rcf[:], pbf[:])
                nc.vector.tensor_mul(ot[jf][0:64, c], fpv0[0:64, c], rcf[0:64, :])
                nc.vector.tensor_mul(ot[jf][64:128, c], fpv1[0:64, c], rcf[64:128, :])
                outproj(jf, 4 * jf + sl)

            # projections for chunk 0 up front
            proj_qk(0, "q")
            proj_qk(0, "k")
            pbv = pp.tile([P, 2, 64], F32, name="pbv", tag="mx", bufs=2)
            nc.tensor.matmul(pbv[:], ones1, bv, start=True, stop=True)
            nc.vector.tensor_copy(bvr[:], pbv[:])
            for i in range(NK):
                nc.gpsimd.memset(v01[i][:, :, 64:65], 1.0)
            for p in range(14):
                nc.gpsimd.memset(v8[p][:, :, :, 64:65], 1.0)

            nonstate = {"nexp": 0}
            for g in range(NJ):
                j = g
                nkj = 4 * (j + 1)
                ppv0 = pp.tile([65, 512], F32, name=f"ppv0_{j}", tag="ppv0", bufs=1)
                ppv1 = pp.tile([65, 512], F32, name=f"ppv1_{j}", tag="ppv1", bufs=1)
                ppv[j] = (ppv0, ppv1)

                # work units spread across this i-loop: projections for chunk
                # g+1 and the out-projection of the already-normalized chunk g-1
                units = []
                if g + 1 < NJ:
                    units.append(lambda g=g: proj_qk(g + 1, "q"))
                if g >= 1:
                    # this chunk's own V tiles: consumed at i >= 4g (the
                    # diagonal); schedule them early so the diag ppv LDWEIGHTS
                    # never waits on the DVE bias-add that produces v01
                    for t in range(4 * g, 4 * g + 2):
                        units.append(lambda t=t: proj_v(t))
                if g + 1 < NJ:
                    units.append(lambda g=g: proj_qk(g + 1, "k"))
                if g >= 1:
                    for t in range(4 * g + 2, 4 * g + 4):
                        units.append(lambda t=t: proj_v(t))
                # out-projection tiles spread in proportion to chunk length so
                # the 2-buffer mx psum pool never has 3 users in flight: chunk
                # g handles OP_SHARE[g] tiles in order (tile t of chunk t//4
                # becomes eligible 2 chunks later; chunk 6's tiles must go in
                # chunk 7 right after normalize(6)).
                for t in range(OP_START[g], OP_START[g] + OP_SHARE[g]):
                    units.append(lambda t=t: outproj(t // 4, t))
                nu = len(units)
                slots = {}
                for u in range(nu):
                    slots.setdefault(min(nkj - 1, 1 + (u * nkj) // (nu + 1)), []).append(units[u])

                # software-pipelined with lag 2+: exp gets >= two block-walls
                # before the in-order PE queue consumes it. Non-diag blocks go
                # through fp8 e + fp8 [V|1] DoubleRow matmuls (two k-tiles per
                # MM); diagonal blocks keep the masked bf16 Schraudolph path.
                # Scores psum bufs=2 suffices: exp(i) completes before
                # scores(i+2) needs its buffer.
                et = {}
                e8s = {}
                # diag blocks consume at lag dlag (3 normally, 1 on the last
                # chunk: the PE has nothing else left, so chase the epilogue);
                # fp8 pairs at ii = 2p+4 so they never wait on a FIFO'd exp.
                # PSUM accumulation is additive, so the j=NJ-1 interleaving of
                # diag blocks before the last pairs is safe (start was pair 0).
                dlag = 1 if j == NJ - 1 else 3
                for ii in range(nkj + dlag):
                    if ii < nkj:
                        i = ii
                        ps = pp.tile([P, 2, 512], F32, name=f"ps_{j}_{i}", tag="s", bufs=2)
                        m = i - 4 * j
                        c0 = 128 * m if m >= 1 else 0
                        nc.tensor.matmul(
                            ps[:, 0, c0:512], kt[i // 4][0:64, ts(i % 4, P)],
                            qt[j][0:64, c0:512], start=True, stop=True,
                        )
                        nc.tensor.matmul(
                            ps[:, 1, c0:512], kt[i // 4][64:128, ts(i % 4, P)],
                            qt[j][64:128, c0:512], start=True, stop=True,
                        )
                        if m >= 0:
                            e = wp.tile([P, 2, 512], BF16, name=f"e_{j}_{i}", tag="e", bufs=6)
                            et[i] = (e, c0)
                            # Schraudolph exp with fused causal mask on DVE
                            nc.vector.scalar_tensor_tensor(
                                e[:, :, c0:512].bitcast(I16), ps[:, :, c0:512],
                                SCA, maskt[:, m, :, c0:512], op0=MUL, op1=ADD,
                            )
                        else:
                            p8 = i // 2
                            if i % 2 == 0:
                                e8s[p8] = wp.tile(
                                    [P, 2, 2, 512], F8E4,
                                    name=f"e8_{j}_{p8}", tag="e8", bufs=4,
                                )
                            e8t = e8s[p8]
                            self_i = nonstate["nexp"]
                            nonstate["nexp"] += 1
                            if self_i % EXPMOD < NACT:
                                nc.scalar.activation(
                                    e8t[:, i % 2, :, :], ps[:], EXP, scale=0.125
                                )
                            else:
                                nc.vector.tensor_scalar(
                                    e8t[:, i % 2, :, :].bitcast(I8), ps[:],
                                    SCA8, SCB8, op0=MUL, op1=ADD,
                                )
                        if j == 0:
                            proj_v(i)
                        if i == 0 and j > 0:
                            normalize(j - 1)
                    if ii >= 4 and ii % 2 == 0 and (ii - 4) // 2 < 2 * j:
                        # DoubleRow pair (2p, 2p+1): fp8 e x fp8 [V|1]
                        p8 = (ii - 4) // 2
                        e8t = e8s.pop(p8)
                        nc.tensor.matmul(
                            ppv0[:], v8[p8][:, :, 0, 0:65], e8t[:, :, 0, :],
                            perf_mode=DR, start=(p8 == 0), stop=False,
                        )
                        nc.tensor.matmul(
                            ppv1[:], v8[p8][:, :, 1, 0:65], e8t[:, :, 1, :],
                            perf_mode=DR, start=(p8 == 0), stop=False,
                        )
                    if ii >= dlag and ii - dlag >= 4 * j:
                        i = ii - dlag
                        e, c0 = et.pop(i)
                        nc.tensor.matmul(
                            ppv0[:, c0:512], v01[i][:, 0, :], e[:, 0, c0:512],
                            start=(i == 0 and j == 0), stop=(i == nkj - 1),
                        )
                        nc.tensor.matmul(
                            ppv1[:, c0:512], v01[i][:, 1, :], e[:, 1, c0:512],
                            start=(i == 0 and j == 0), stop=(i == nkj - 1),
                        )
                        # last chunk: with column restriction, ppv columns
                        # [128*sl, 128*sl+128) are final once iteration
                        # i = 4*j + sl has accumulated -> stream the epilogue
                        # slice-by-slice instead of serializing it at the end
                        if j == NJ - 1 and 0 <= i - 4 * j < 4:
                            final_slice(i - 4 * j)
                    for fn in slots.get(ii, []):
                        fn()

    nc.compile()
    return nc


def _pack_w(w_slice, dt=ml_dtypes.bfloat16):
    """[HDC, D] weight slice -> transposed, chunked [P, KO, HDC]."""
    wt = np.ascontiguousarray(w_slice.T)          # [D, HDC]
    return np.ascontiguousarray(
        wt.reshape(KO, P, HDC).transpose(1, 0, 2)
    ).astype(dt)


def _make_in_maps(x, W_qkv, b_qkv, W_out, b_out):
    bf = ml_dtypes.bfloat16
    x = np.asarray(x, np.float32)
    W_qkv = np.asarray(W_qkv, np.float32)
    b_qkv = np.asarray(b_qkv, np.float32)
    W_out = np.asarray(W_out, np.float32)
    xt = np.ascontiguousarray(
        x.T.reshape(KO, P, NJ, 512).transpose(1, 2, 0, 3)
    ).astype(bf)
    in_maps = []
    for c in range(NCORES):
        r = slice(HDC * c, HDC * (c + 1))
        in_maps.append({
            "xt": xt,
            "wq": _pack_w(W_qkv[0 * D:1 * D][r]),
            "wk": _pack_w(W_qkv[1 * D:2 * D][r]),
            "wv": _pack_w(W_qkv[2 * D:3 * D][r]),
            "wo": np.ascontiguousarray(W_out[:, r].T).astype(bf),
            "bqk": np.ascontiguousarray(
                np.stack(
                    [b_qkv[0 * D:1 * D][r], b_qkv[1 * D:2 * D][r]], axis=1
                )
            ).astype(np.float32),
            "cst": np.ascontiguousarray(
                np.concatenate(
                    [b_qkv[2 * D:3 * D][r], np.ones(192, np.float32)]
                )[None, :]
            ).astype(bf),
        })
    return in_maps


_NC_CACHE = {}


def kernel(x, W_qkv, b_qkv, W_out, b_out):
    if "nc" not in _NC_CACHE:
        _NC_CACHE["nc"] = _build()
    nc = _NC_CACHE["nc"]
    in_maps = _make_in_maps(x, W_qkv, b_qkv, W_out, b_out)
    res = run_bass_kernel_spmd(nc, in_maps, core_ids=list(range(NCORES)))
    out = np.zeros((L, D), np.float32)
    for c in range(NCORES):
        out += res.results[c]["out"].astype(np.float32)
    out += np.asarray(b_out, np.float32)[None, :]
    return out



# revision 44
# speedup vs baseline: 1.0153x; 1.0009x over previous
"""Causal self-attention (L=4096, D=1024, 16 heads) on 8 TRN2 NeuronCores.

Sharding: tensor-parallel over heads — each core owns 2 heads (128 head-dims).
Per core:
  QT/KT = W @ x.T (+bias)          [128, L]   (head-dims on partitions)
  V     = x @ Wv.T (+bias)         [L, 128]   (tokens on partitions)
  S.T   = K @ Q.T  (per head)      [k, q] blocks, causal-skipped
  E     = exp(S.T/8) * mask        (no max-subtraction: |logits| < ~3.1)
  O.T   = [V|1].T @ E              -> unnormalized head outputs + col-sums
  O.T  /= sums  (PE broadcast + DVE reciprocal)
  partial = O @ Wo_slice.T         [L, D]
Host: out = sum_c(partial_c) + b_out.

All matmuls bf16 with fp32 PSUM accumulation. The exp is split across two
engines: ACT runs true exp; the DVE runs a Schraudolph bf16 approximation
(int16 affine of the raw scores whose bits ARE bf16(exp)), which also fuses
the causal mask via a per-element bias tile that drops masked entries into
bf16-denormal territory. Diagonal blocks restrict scores/exp/ppv to the
causally reachable column range. ppv is software-pipelined one iteration
behind exp so the in-order PE queue never waits on a just-launched exp, and
projection work runs two chunks ahead. Measured rel err ~1.1e-2.
"""

import numpy as np
import ml_dtypes

import concourse.bass as bass
import concourse.mybir as mybir
import concourse.tile as tile
from concourse import bacc
from concourse.bass import ts
from concourse.bass_utils import run_bass_kernel_spmd

L, D = 4096, 1024
P = 128
NCORES = 8
HDC = 128          # head-dims per core (2 heads x 64)
KO = D // P        # 8 contraction chunks of the model dim
NJ = L // 512      # 8 q-chunks of 512
NK = L // P        # 32 k-chunks of 128
BF16 = mybir.dt.bfloat16
F32 = mybir.dt.float32
F8E4 = mybir.dt.float8e4
I16 = mybir.dt.int16
I8 = mybir.dt.int8
EXP = mybir.ActivationFunctionType.Exp
IDENT = mybir.ActivationFunctionType.Identity
MUL = mybir.AluOpType.mult
ADD = mybir.AluOpType.add
DR = mybir.MatmulPerfMode.DoubleRow

# Schraudolph bf16 exp: bits_i16 = SCA*s + SCB approximates bf16(exp(s/8)).
# SCA = 128/ln(2)/8; SCB = 127*128 - 5.59 (mantissa-linearization shift).
# Masked positions add -15600 so bits land in [0, ~1300) -> bf16 denormal ~ 0.
SCA = 23.083120654223414
SCB = 16250.41
SCMASK = -15600.0
# Schraudolph fp8-e4m3 exp for the DoubleRow path: bits_i8 = SCA8*s + SCB8
# approximates the e4m3 bit pattern of exp(s/8) (3-bit mantissa).
SCA8 = 8.0 / (8.0 * float(np.log(2.0)))
SCB8 = 56.0 - 0.45
# non-diag exp engine mix: i % EXPMOD in ACT_SLOTS -> ACT, else DVE schr8
EXPMOD = 10
NACT = 7
# outproj tiles handled per chunk (prefix sums give each chunk's start tile)
OP_SHARE = [0, 0, 4, 4, 4, 4, 4, 8]
OP_START = [0, 0, 0, 4, 8, 12, 16, 20]


def _build():
    nc = bacc.Bacc("TRN2", target_bir_lowering=False)

    xt_d = nc.dram_tensor("xt", [P, NJ, KO, 512], BF16, kind="ExternalInput")
    wq_d = nc.dram_tensor("wq", [P, KO, HDC], BF16, kind="ExternalInput")
    wk_d = nc.dram_tensor("wk", [P, KO, HDC], BF16, kind="ExternalInput")
    wv_d = nc.dram_tensor("wv", [P, KO, HDC], BF16, kind="ExternalInput")
    wo_d = nc.dram_tensor("wo", [HDC, D], BF16, kind="ExternalInput")
    bqk_d = nc.dram_tensor("bqk", [HDC, 2], F32, kind="ExternalInput")
    # [bv | ones128 | ones64] packed into one row-tensor
    cst_d = nc.dram_tensor("cst", [1, 320], BF16, kind="ExternalInput")
    out_d = nc.dram_tensor("out", [L, D], BF16, kind="ExternalOutput")

    # Schraudolph bias-with-mask for the 4 diagonal-block offsets, per head:
    # bmask[p, m, h, c] = SCB if c >= p + 128*m else SCB + SCMASK (pushes the
    # int16 bits into bf16-denormal land -> e ~= 0). Shipped as int16 (exact)
    # to halve the startup DMA bytes; the DVE converts to fp32 on read.
    qi = np.arange(512)
    half = (
        qi[None, None, :] >= (np.arange(P)[:, None, None] + 128 * np.arange(4)[None, :, None])
    )
    mask_np = np.stack([half, half], axis=2)          # [P, 4, 2, 512]
    bmask_np = np.where(mask_np, np.int16(round(SCB)), np.int16(round(SCB + SCMASK)))
    mask_d = nc.inline_tensor(np.ascontiguousarray(bmask_np.astype(np.int16)), name="maskc")

    with tile.TileContext(nc) as tc:
        with (
            tc.tile_pool(name="const", bufs=1) as cp,
            tc.tile_pool(name="work", bufs=4) as wp,
            tc.tile_pool(name="psum", bufs=1, space="PSUM") as pp,
        ):
            # ---- first-needed DMAs first: wq/wk + token-chunk-0/1 of x gate
            # the first projections; everything else is deferred behind them.
            wq = cp.tile([P, KO, HDC], BF16, name="wq_s", tag="wq_s")
            wk = cp.tile([P, KO, HDC], BF16, name="wk_s", tag="wk_s")
            wv = cp.tile([P, KO, HDC], BF16, name="wv_s", tag="wv_s")
            xta = cp.tile([P, NJ, KO, 512], BF16, name="xta", tag="xta")
            maskt = cp.tile([P, 4, 2, 512], I16, name="mask_s", tag="mask_s")
            wo = cp.tile([P, D], BF16, name="wo_s", tag="wo_s")
            bqk = cp.tile([P, 2], F32, name="bqk_s", tag="bqk_s")
            cst = cp.tile([1, 320], BF16, name="cst_s", tag="cst_s")
            bv = cst[0:1, 0:128]
            ones1 = cst[0:1, 128:256]
            ones64 = cst[0:1, 256:320]
            bq = bqk[:, 0:1]
            bk = bqk[:, 1:2]
            # first-needed pieces lead each HW DMA queue so the first proj
            # matmuls (fp8 wq x xt8[:,0]) can start as early as possible; the
            # mask (for j=0 diag exp) streams per-m right behind them.
            nc.scalar.dma_start(wq[:, 0:1], wq_d[:, 0:1])
            nc.sync.dma_start(xta[:, 0, 0:1], xt_d[:, 0, 0:1])
            nc.scalar.dma_start(wq[:, 1:8], wq_d[:, 1:8])
            nc.sync.dma_start(xta[:, 0, 1:5], xt_d[:, 0, 1:5])
            nc.scalar.dma_start(wk[:], wk_d[:])
            nc.sync.dma_start(xta[:, 0, 5:8], xt_d[:, 0, 5:8])
            nc.scalar.dma_start(bqk[:], bqk_d[:])
            nc.sync.dma_start(maskt[:, 0:2], mask_d[:, 0:2])
            nc.scalar.dma_start(wv[:], wv_d[:])
            nc.scalar.dma_start(cst[:], cst_d[:])
            nc.sync.dma_start(maskt[:, 2:4], mask_d[:, 2:4])
            nc.scalar.dma_start(xta[:, 1, 0:4], xt_d[:, 1, 0:4])
            nc.scalar.dma_start(xta[:, 1, 4:8], xt_d[:, 1, 4:8])
            nc.sync.dma_start(xta[:, 2, 0:4], xt_d[:, 2, 0:4])
            nc.sync.dma_start(xta[:, 2, 4:8], xt_d[:, 2, 4:8])
            nc.sync.dma_start(wo[:], wo_d[:])
            for jcol in range(3, NJ):
                if jcol % 2 == 1:
                    nc.scalar.dma_start(xta[:, jcol], xt_d[:, jcol])
                else:
                    nc.sync.dma_start(xta[:, jcol], xt_d[:, jcol])

            # HAM warm-up: ~3.5us of throwaway matmuls on memset scratch while
            # the first DMAs are in flight, so the real matmuls start at the
            # warm 2.4 GHz clock instead of the cold 1.2 GHz default. The
            # memset goes first on the (empty) gpsimd queue.
            wsc = cp.tile([P, 512], BF16, name="wsc", tag="wsc")
            nc.gpsimd.memset(wsc[:], 0.0)
            wps = pp.tile([P, 512], F32, name="wps", tag="mx", bufs=2)
            for _ in range(8):
                nc.tensor.matmul(wps[:], wsc[:, 0:128], wsc[:], start=True, stop=True)

            qt = [cp.tile([P, 512], BF16, name=f"qt{j}", tag=f"qt{j}") for j in range(NJ)]
            kt = [cp.tile([P, 512], BF16, name=f"kt{j}", tag=f"kt{j}") for j in range(NJ)]
            ot = [cp.tile([P, 512], BF16, name=f"ot{j}", tag=f"ot{j}") for j in range(NJ)]
            v01 = [cp.tile([P, 2, 65], BF16, name=f"v01_{i}", tag=f"v01_{i}") for i in range(NK)]
            bvr = cp.tile([P, 2, 64], BF16, name="bvr_s", tag="bvr_s")
            # fp8 [V|1] pair tiles for DoubleRow ppv: [slot(2), head(2), 80]
            # (pair p packs k-tiles 2p/2p+1; tiles 28..31 are diag-only)
            v8 = [
                cp.tile([P, 2, 2, 80], F8E4, name=f"v8_{p}", tag=f"v8_{p}")
                for p in range(14)
            ]

            ppv = {}  # j -> (ppv0, ppv1) accumulation psums kept until epilogue

            def proj_qk(g, which):
                """QT or KT projection for token chunk g (bias add: q on ACT,
                k on DVE, for engine balance)."""
                w, b, dst, nm = (wq, bq, qt, "q") if which == "q" else (wk, bk, kt, "k")
                ps = pp.tile([P, 512], F32, name=f"ps{nm}{g}", tag="mx", bufs=2)
                for k in range(KO):
                    nc.tensor.matmul(
                        ps[:], w[:, k, :], xta[:, g, k, :],
                        start=(k == 0), stop=(k == KO - 1),
                    )
                if which == "q":
                    nc.scalar.activation(dst[g][:], ps[:], IDENT, bias=b)
                else:
                    nc.vector.tensor_scalar_add(dst[g][:], ps[:], b)

            def proj_v(t):
                """V projection for token tile t (both heads + bias + ones col)."""
                psv = pp.tile([P, 2, 64], F32, name=f"psv{t}", tag="mx", bufs=2)
                for k in range(KO):
                    nc.tensor.matmul(
                        psv[:], xta[:, t // 4, k, ts(t % 4, P)], wv[:, k, :],
                        start=(k == 0), stop=(k == KO - 1),
                    )
                nc.vector.tensor_tensor(
                    v01[t][:, :, 0:64], psv[:], bvr[:], mybir.AluOpType.add
                )
                if t < 28:
                    # fp8 copy for the DoubleRow path (gpsimd is otherwise idle)
                    nc.gpsimd.tensor_copy(
                        v8[t // 2][:, t % 2, :, 0:64], v01[t][:, :, 0:64]
                    )

            def normalize(j):
                """Normalize chunk j's head outputs into ot[j]."""
                ppv0, ppv1 = ppv.pop(j)
                s0 = wp.tile([1, 512], BF16, name=f"s0_{j}", tag="s0", bufs=4)
                s1 = wp.tile([1, 512], BF16, name=f"s1_{j}", tag="s1", bufs=4)
                nc.scalar.copy(s0[:], ppv0[64:65, :])
                nc.vector.tensor_copy(s1[:], ppv1[64:65, :])
                pb = pp.tile([P, 512], F32, name=f"pb_{j}", tag="mx", bufs=2)
                nc.tensor.matmul(pb[0:64, :], ones64, s0[:], start=True, stop=True,
                                 tile_position=(0, 0))
                nc.tensor.matmul(pb[64:128, :], ones64, s1[:], start=True, stop=True,
                                 tile_position=(0, 64))
                rc = wp.tile([P, 512], F32, name=f"rc_{j}", tag="rc", bufs=4)
                nc.vector.reciprocal_approx_fast(rc[:], pb[:])
                nc.vector.tensor_mul(ot[j][0:64, :], ppv0[0:64, :], rc[0:64, :])
                nc.vector.tensor_mul(ot[j][64:128, :], ppv1[0:64, :], rc[64:128, :])

            def outproj(j, t):
                ob = wp.tile([P, D], BF16, name=f"ob_{t}", tag="ob", bufs=8)
                for n in range(2):
                    po = pp.tile([P, 512], F32, name=f"po_{t}_{n}", tag="mx", bufs=2)
                    nc.tensor.matmul(
                        po[:], ot[j][:, ts(t - 4 * j, P)], wo[:, ts(n, 512)],
                        start=True, stop=True,
                    )
                    # each half DMAs right after its own copy, on its own queue
                    if n == 1:
                        nc.scalar.copy(ob[:, ts(n, 512)], po[:])
                        nc.scalar.dma_start(out_d[ts(t, P), ts(n, 512)], ob[:, ts(n, 512)])
                    else:
                        nc.vector.tensor_copy(ob[:, ts(n, 512)], po[:])
                        nc.sync.dma_start(out_d[ts(t, P), ts(n, 512)], ob[:, ts(n, 512)])

            jf = NJ - 1

            def final_slice(sl):
                """Normalize + out-project one 128-token slice of the last
                chunk (its ppv columns are final after iteration 4*jf+sl)."""
                fpv0, fpv1 = ppv[jf]
                c = ts(sl, P)
                s0 = wp.tile([1, P], BF16, name=f"s0f_{sl}", tag="s0f", bufs=4)
                s1 = wp.tile([1, P], BF16, name=f"s1f_{sl}", tag="s1f", bufs=4)
                nc.scalar.copy(s0[:], fpv0[64:65, c])
                nc.vector.tensor_copy(s1[:], fpv1[64:65, c])
                pbf = pp.tile([P, P], F32, name=f"pbf_{sl}", tag="mx", bufs=2)
                nc.tensor.matmul(pbf[0:64, :], ones64, s0[:], start=True, stop=True,
                                 tile_position=(0, 0))
                nc.tensor.matmul(pbf[64:128, :], ones64, s1[:], start=True, stop=True,
                                 tile_position=(0, 64))
                rcf = wp.tile([P, P], F32, name=f"rcf_{sl}", tag="rcf", bufs=4)
                nc.vector.reciprocal_approx_fast(rcf[:], pbf[:])
                nc.vector.tensor_mul(ot[jf][0:64, c], fpv0[0:64, c], rcf[0:64, :])
                nc.vector.tensor_mul(ot[jf][64:128, c], fpv1[0:64, c], rcf[64:128, :])
                outproj(jf, 4 * jf + sl)

            # projections for chunk 0 up front
            proj_qk(0, "q")
            proj_qk(0, "k")
            pbv = pp.tile([P, 2, 64], F32, name="pbv", tag="mx", bufs=2)
            nc.tensor.matmul(pbv[:], ones1, bv, start=True, stop=True)
            nc.vector.tensor_copy(bvr[:], pbv[:])
            for i in range(NK):
                nc.gpsimd.memset(v01[i][:, :, 64:65], 1.0)
            for p in range(14):
                nc.gpsimd.memset(v8[p][:, :, :, 64:65], 1.0)

            nonstate = {"nexp": 0}
            for g in range(NJ):
                j = g
                nkj = 4 * (j + 1)
                ppv0 = pp.tile([65, 512], F32, name=f"ppv0_{j}", tag="ppv0", bufs=1)
                ppv1 = pp.tile([65, 512], F32, name=f"ppv1_{j}", tag="ppv1", bufs=1)
                ppv[j] = (ppv0, ppv1)

                # work units spread across this i-loop: projections for chunk
                # g+1 and the out-projection of the already-normalized chunk g-1
                units = []
                if g + 1 < NJ:
                    units.append(lambda g=g: proj_qk(g + 1, "q"))
                if g >= 1:
                    # this chunk's own V tiles: consumed at i >= 4g (the
                    # diagonal); schedule them early so the diag ppv LDWEIGHTS
                    # never waits on the DVE bias-add that produces v01
                    for t in range(4 * g, 4 * g + 2):
                        units.append(lambda t=t: proj_v(t))
                if g + 1 < NJ:
                    units.append(lambda g=g: proj_qk(g + 1, "k"))
                if g >= 1:
                    for t in range(4 * g + 2, 4 * g + 4):
                        units.append(lambda t=t: proj_v(t))
                # out-projection tiles spread in proportion to chunk length so
                # the 2-buffer mx psum pool never has 3 users in flight: chunk
                # g handles OP_SHARE[g] tiles in order (tile t of chunk t//4
                # becomes eligible 2 chunks later; chunk 6's tiles must go in
                # chunk 7 right after normalize(6)).
                for t in range(OP_START[g], OP_START[g] + OP_SHARE[g]):
                    units.append(lambda t=t: outproj(t // 4, t))
                nu = len(units)
                slots = {}
                for u in range(nu):
                    slots.setdefault(min(nkj - 1, 1 + (u * nkj) // (nu + 1)), []).append(units[u])

                # software-pipelined with lag 2+: exp gets >= two block-walls
                # before the in-order PE queue consumes it. Non-diag blocks go
                # through fp8 e + fp8 [V|1] DoubleRow matmuls (two k-tiles per
                # MM); diagonal blocks keep the masked bf16 Schraudolph path.
                # Scores psum bufs=2 suffices: exp(i) completes before
                # scores(i+2) needs its buffer.
                et = {}
                e8s = {}
                # diag blocks consume at lag dlag (3 normally, 1 on the last
                # chunk: the PE has nothing else left, so chase the epilogue);
                # fp8 pairs at ii = 2p+4 so they never wait on a FIFO'd exp.
                # PSUM accumulation is additive, so the j=NJ-1 interleaving of
                # diag blocks before the last pairs is safe (start was pair 0).
                dlag = 1 if j == NJ - 1 else 3
                for ii in range(nkj + dlag):
                    if ii < nkj:
                        i = ii
                        ps = pp.tile([P, 2, 512], F32, name=f"ps_{j}_{i}", tag="s", bufs=2)
                        m = i - 4 * j
                        c0 = 128 * m if m >= 1 else 0
                        nc.tensor.matmul(
                            ps[:, 0, c0:512], kt[i // 4][0:64, ts(i % 4, P)],
                            qt[j][0:64, c0:512], start=True, stop=True,
                        )
                        nc.tensor.matmul(
                            ps[:, 1, c0:512], kt[i // 4][64:128, ts(i % 4, P)],
                            qt[j][64:128, c0:512], start=True, stop=True,
                        )
                        if m >= 0:
                            e = wp.tile([P, 2, 512], BF16, name=f"e_{j}_{i}", tag="e", bufs=6)
                            et[i] = (e, c0)
                            # Schraudolph exp with fused causal mask on DVE
                            nc.vector.scalar_tensor_tensor(
                                e[:, :, c0:512].bitcast(I16), ps[:, :, c0:512],
                                SCA, maskt[:, m, :, c0:512], op0=MUL, op1=ADD,
                            )
                        else:
                            p8 = i // 2
                            if i % 2 == 0:
                                e8s[p8] = wp.tile(
                                    [P, 2, 2, 512], F8E4,
                                    name=f"e8_{j}_{p8}", tag="e8", bufs=4,
                                )
                            e8t = e8s[p8]
                            self_i = nonstate["nexp"]
                            nonstate["nexp"] += 1
                            if self_i % EXPMOD < NACT:
                                nc.scalar.activation(
                                    e8t[:, i % 2, :, :], ps[:], EXP, scale=0.125
                                )
                            else:
                                nc.vector.tensor_scalar(
                                    e8t[:, i % 2, :, :].bitcast(I8), ps[:],
                                    SCA8, SCB8, op0=MUL, op1=ADD,
                                )
                        if j == 0:
                            proj_v(i)
                        if i == 0 and j > 0:
                            normalize(j - 1)
                    if ii >= 4 and ii % 2 == 0 and (ii - 4) // 2 < 2 * j:
                        # DoubleRow pair (2p, 2p+1): fp8 e x fp8 [V|1]
                        p8 = (ii - 4) // 2
                        e8t = e8s.pop(p8)
                        nc.tensor.matmul(
                            ppv0[:], v8[p8][:, :, 0, 0:65], e8t[:, :, 0, :],
                            perf_mode=DR, start=(p8 == 0), stop=False,
                        )
                        nc.tensor.matmul(
                            ppv1[:], v8[p8][:, :, 1, 0:65], e8t[:, :, 1, :],
                            perf_mode=DR, start=(p8 == 0), stop=False,
                        )
                    if ii >= dlag and ii - dlag >= 4 * j:
                        i = ii - dlag
                        e, c0 = et.pop(i)
                        nc.tensor.matmul(
                            ppv0[:, c0:512], v01[i][:, 0, :], e[:, 0, c0:512],
                            start=(i == 0 and j == 0), stop=(i == nkj - 1),
                        )
                        nc.tensor.matmul(
                            ppv1[:, c0:512], v01[i][:, 1, :], e[:, 1, c0:512],
                            start=(i == 0 and j == 0), stop=(i == nkj - 1),
                        )
                        # last chunk: with column restriction, ppv columns
                        # [128*sl, 128*sl+128) are final once iteration
                        # i = 4*j + sl has accumulated -> stream the epilogue
                        # slice-by-slice instead of serializing it at the end
                        if j == NJ - 1 and 0 <= i - 4 * j < 4:
                            final_slice(i - 4 * j)
                    for fn in slots.get(ii, []):
                        fn()

    nc.compile()
    return nc


def _pack_w(w_slice, dt=ml_dtypes.bfloat16):
    """[HDC, D] weight slice -> transposed, chunked [P, KO, HDC]."""
    wt = np.ascontiguousarray(w_slice.T)          # [D, HDC]
    return np.ascontiguousarray(
        wt.reshape(KO, P, HDC).transpose(1, 0, 2)
    ).astype(dt)


def _make_in_maps(x, W_qkv, b_qkv, W_out, b_out):
    bf = ml_dtypes.bfloat16
    x = np.asarray(x, np.float32)
    W_qkv = np.asarray(W_qkv, np.float32)
    b_qkv = np.asarray(b_qkv, np.float32)
    W_out = np.asarray(W_out, np.float32)
    xt = np.ascontiguousarray(
        x.T.reshape(KO, P, NJ, 512).transpose(1, 2, 0, 3)
    ).astype(bf)
    in_maps = []
    for c in range(NCORES):
        r = slice(HDC * c, HDC * (c + 1))
        in_maps.append({
            "xt": xt,
            "wq": _pack_w(W_qkv[0 * D:1 * D][r]),
            "wk": _pack_w(W_qkv[1 * D:2 * D][r]),
            "wv": _pack_w(W_qkv[2 * D:3 * D][r]),
            "wo": np.ascontiguousarray(W_out[:, r].T).astype(bf),
            "bqk": np.ascontiguousarray(
                np.stack(
                    [b_qkv[0 * D:1 * D][r], b_qkv[1 * D:2 * D][r]], axis=1
                )
            ).astype(np.float32),
            "cst": np.ascontiguousarray(
                np.concatenate(
                    [b_qkv[2 * D:3 * D][r], np.ones(192, np.float32)]
                )[None, :]
            ).astype(bf),
        })
    return in_maps


_NC_CACHE = {}


def kernel(x, W_qkv, b_qkv, W_out, b_out):
    if "nc" not in _NC_CACHE:
        _NC_CACHE["nc"] = _build()
    nc = _NC_CACHE["nc"]
    in_maps = _make_in_maps(x, W_qkv, b_qkv, W_out, b_out)
    res = run_bass_kernel_spmd(nc, in_maps, core_ids=list(range(NCORES)))
    out = np.zeros((L, D), np.float32)
    for c in range(NCORES):
        out += res.results[c]["out"].astype(np.float32)
    out += np.asarray(b_out, np.float32)[None, :]
    return out



# revision 45
# speedup vs baseline: 1.0392x; 1.0235x over previous
"""Causal self-attention (L=4096, D=1024, 16 heads) on 8 TRN2 NeuronCores.

Sharding: tensor-parallel over heads — each core owns 2 heads (128 head-dims).
Per core:
  QT/KT = W @ x.T (+bias)          [128, L]   (head-dims on partitions)
  V     = x @ Wv.T (+bias)         [L, 128]   (tokens on partitions)
  S.T   = K @ Q.T  (per head)      [k, q] blocks, causal-skipped
  E     = exp(S.T/8) * mask        (no max-subtraction: |logits| < ~3.1)
  O.T   = [V|1].T @ E              -> unnormalized head outputs + col-sums
  O.T  /= sums  (PE broadcast + DVE reciprocal)
  partial = O @ Wo_slice.T         [L, D]
Host: out = sum_c(partial_c) + b_out.

All matmuls bf16 with fp32 PSUM accumulation. The exp is split across two
engines: ACT runs true exp; the DVE runs a Schraudolph bf16 approximation
(int16 affine of the raw scores whose bits ARE bf16(exp)), which also fuses
the causal mask via a per-element bias tile that drops masked entries into
bf16-denormal territory. Diagonal blocks restrict scores/exp/ppv to the
causally reachable column range. ppv is software-pipelined one iteration
behind exp so the in-order PE queue never waits on a just-launched exp, and
projection work runs two chunks ahead. Measured rel err ~1.1e-2.
"""

import numpy as np
import ml_dtypes

import concourse.bass as bass
import concourse.mybir as mybir
import concourse.tile as tile
from concourse import bacc
from concourse.bass import ts
from concourse.bass_utils import run_bass_kernel_spmd

L, D = 4096, 1024
P = 128
NCORES = 8
HDC = 128          # head-dims per core (2 heads x 64)
KO = D // P        # 8 contraction chunks of the model dim
NJ = L // 512      # 8 q-chunks of 512
NK = L // P        # 32 k-chunks of 128
BF16 = mybir.dt.bfloat16
F32 = mybir.dt.float32
F8E4 = mybir.dt.float8e4
I16 = mybir.dt.int16
I8 = mybir.dt.int8
EXP = mybir.ActivationFunctionType.Exp
IDENT = mybir.ActivationFunctionType.Identity
MUL = mybir.AluOpType.mult
ADD = mybir.AluOpType.add
DR = mybir.MatmulPerfMode.DoubleRow

# Schraudolph bf16 exp: bits_i16 = SCA*s + SCB approximates bf16(exp(s/8)).
# SCA = 128/ln(2)/8; SCB = 127*128 - 5.59 (mantissa-linearization shift).
# Masked positions add -15600 so bits land in [0, ~1300) -> bf16 denormal ~ 0.
SCA = 23.083120654223414
SCB = 16250.41
SCMASK = -15600.0
# Schraudolph fp8-e4m3 exp for the DoubleRow path: bits_i8 = SCA8*s + SCB8
# approximates the e4m3 bit pattern of exp(s/8) (3-bit mantissa).
SCA8 = 8.0 / (8.0 * float(np.log(2.0)))
SCB8 = 56.0 - 0.45
# non-diag exp engine mix: i % EXPMOD in ACT_SLOTS -> ACT, else DVE schr8
EXPMOD = 3
NACT = 2
# outproj tiles handled per chunk (prefix sums give each chunk's start tile)
OP_SHARE = [0, 0, 4, 4, 4, 4, 4, 8]
OP_START = [0, 0, 0, 4, 8, 12, 16, 20]


def _build():
    nc = bacc.Bacc("TRN2", target_bir_lowering=False)

    xt_d = nc.dram_tensor("xt", [P, NJ, KO, 512], BF16, kind="ExternalInput")
    wq_d = nc.dram_tensor("wq", [P, KO, HDC], BF16, kind="ExternalInput")
    wk_d = nc.dram_tensor("wk", [P, KO, HDC], BF16, kind="ExternalInput")
    wv_d = nc.dram_tensor("wv", [P, KO, HDC], BF16, kind="ExternalInput")
    wo_d = nc.dram_tensor("wo", [HDC, D], BF16, kind="ExternalInput")
    bqk_d = nc.dram_tensor("bqk", [HDC, 2], F32, kind="ExternalInput")
    # [bv | ones128 | ones64] packed into one row-tensor
    cst_d = nc.dram_tensor("cst", [1, 320], BF16, kind="ExternalInput")
    out_d = nc.dram_tensor("out", [L, D], BF16, kind="ExternalOutput")

    # Schraudolph bias-with-mask for the 4 diagonal-block offsets, per head:
    # bmask[p, m, h, c] = SCB if c >= p + 128*m else SCB + SCMASK (pushes the
    # int16 bits into bf16-denormal land -> e ~= 0). Shipped as int16 (exact)
    # to halve the startup DMA bytes; the DVE converts to fp32 on read.
    qi = np.arange(512)
    half = (
        qi[None, None, :] >= (np.arange(P)[:, None, None] + 128 * np.arange(4)[None, :, None])
    )
    mask_np = np.stack([half, half], axis=2)          # [P, 4, 2, 512]
    bmask_np = np.where(mask_np, np.int16(round(SCB)), np.int16(round(SCB + SCMASK)))
    mask_d = nc.inline_tensor(np.ascontiguousarray(bmask_np.astype(np.int16)), name="maskc")

    with tile.TileContext(nc) as tc:
        with (
            tc.tile_pool(name="const", bufs=1) as cp,
            tc.tile_pool(name="work", bufs=4) as wp,
            tc.tile_pool(name="psum", bufs=1, space="PSUM") as pp,
        ):
            # ---- first-needed DMAs first: wq/wk + token-chunk-0/1 of x gate
            # the first projections; everything else is deferred behind them.
            wq = cp.tile([P, KO, HDC], BF16, name="wq_s", tag="wq_s")
            wk = cp.tile([P, KO, HDC], BF16, name="wk_s", tag="wk_s")
            wv = cp.tile([P, KO, HDC], BF16, name="wv_s", tag="wv_s")
            xta = cp.tile([P, NJ, KO, 512], BF16, name="xta", tag="xta")
            maskt = cp.tile([P, 4, 2, 512], I16, name="mask_s", tag="mask_s")
            wo = cp.tile([P, D], BF16, name="wo_s", tag="wo_s")
            bqk = cp.tile([P, 2], F32, name="bqk_s", tag="bqk_s")
            cst = cp.tile([1, 320], BF16, name="cst_s", tag="cst_s")
            bv = cst[0:1, 0:128]
            ones1 = cst[0:1, 128:256]
            ones64 = cst[0:1, 256:320]
            bq = bqk[:, 0:1]
            bk = bqk[:, 1:2]
            # first-needed pieces lead each HW DMA queue so the first proj
            # matmuls (fp8 wq x xt8[:,0]) can start as early as possible; the
            # mask (for j=0 diag exp) streams per-m right behind them.
            nc.scalar.dma_start(wq[:, 0:1], wq_d[:, 0:1])
            nc.sync.dma_start(xta[:, 0, 0:1], xt_d[:, 0, 0:1])
            nc.scalar.dma_start(wq[:, 1:8], wq_d[:, 1:8])
            nc.sync.dma_start(xta[:, 0, 1:3], xt_d[:, 0, 1:3])
            nc.scalar.dma_start(xta[:, 0, 3:5], xt_d[:, 0, 3:5])
            nc.sync.dma_start(xta[:, 0, 5:7], xt_d[:, 0, 5:7])
            nc.scalar.dma_start(wk[:], wk_d[:])
            nc.sync.dma_start(xta[:, 0, 7:8], xt_d[:, 0, 7:8])
            nc.scalar.dma_start(bqk[:], bqk_d[:])
            nc.sync.dma_start(maskt[:, 0:2], mask_d[:, 0:2])
            nc.scalar.dma_start(wv[:], wv_d[:])
            nc.scalar.dma_start(cst[:], cst_d[:])
            nc.sync.dma_start(maskt[:, 2:4], mask_d[:, 2:4])
            nc.scalar.dma_start(xta[:, 1, 0:4], xt_d[:, 1, 0:4])
            nc.scalar.dma_start(xta[:, 1, 4:8], xt_d[:, 1, 4:8])
            nc.sync.dma_start(xta[:, 2, 0:4], xt_d[:, 2, 0:4])
            nc.sync.dma_start(xta[:, 2, 4:8], xt_d[:, 2, 4:8])
            nc.sync.dma_start(wo[:], wo_d[:])
            for jcol in range(3, NJ):
                if jcol % 2 == 1:
                    nc.scalar.dma_start(xta[:, jcol], xt_d[:, jcol])
                else:
                    nc.sync.dma_start(xta[:, jcol], xt_d[:, jcol])

            # HAM warm-up: ~3.5us of throwaway matmuls on memset scratch while
            # the first DMAs are in flight, so the real matmuls start at the
            # warm 2.4 GHz clock instead of the cold 1.2 GHz default. The
            # memset goes first on the (empty) gpsimd queue.
            wsc = cp.tile([P, 512], BF16, name="wsc", tag="wsc")
            nc.gpsimd.memset(wsc[:], 0.0)
            wps = pp.tile([P, 512], F32, name="wps", tag="mx", bufs=2)
            for _ in range(8):
                nc.tensor.matmul(wps[:], wsc[:, 0:128], wsc[:], start=True, stop=True)

            qt = [cp.tile([P, 512], BF16, name=f"qt{j}", tag=f"qt{j}") for j in range(NJ)]
            kt = [cp.tile([P, 512], BF16, name=f"kt{j}", tag=f"kt{j}") for j in range(NJ)]
            ot = [cp.tile([P, 512], BF16, name=f"ot{j}", tag=f"ot{j}") for j in range(NJ)]
            v01 = [cp.tile([P, 2, 65], BF16, name=f"v01_{i}", tag=f"v01_{i}") for i in range(NK)]
            bvr = cp.tile([P, 2, 64], BF16, name="bvr_s", tag="bvr_s")
            # fp8 [V|1] pair tiles for DoubleRow ppv: [slot(2), head(2), 80]
            # (pair p packs k-tiles 2p/2p+1; tiles 28..31 are diag-only)
            v8 = [
                cp.tile([P, 2, 2, 80], F8E4, name=f"v8_{p}", tag=f"v8_{p}")
                for p in range(14)
            ]

            ppv = {}  # j -> (ppv0, ppv1) accumulation psums kept until epilogue

            def proj_qk(g, which):
                """QT or KT projection for token chunk g (bias add: q on ACT,
                k on DVE, for engine balance)."""
                w, b, dst, nm = (wq, bq, qt, "q") if which == "q" else (wk, bk, kt, "k")
                ps = pp.tile([P, 512], F32, name=f"ps{nm}{g}", tag="mx", bufs=2)
                for k in range(KO):
                    nc.tensor.matmul(
                        ps[:], w[:, k, :], xta[:, g, k, :],
                        start=(k == 0), stop=(k == KO - 1),
                    )
                if which == "q":
                    nc.scalar.activation(dst[g][:], ps[:], IDENT, bias=b)
                else:
                    nc.vector.tensor_scalar_add(dst[g][:], ps[:], b)

            def proj_v(t):
                """V projection for token tile t (both heads + bias + ones col)."""
                psv = pp.tile([P, 2, 64], F32, name=f"psv{t}", tag="mx", bufs=2)
                for k in range(KO):
                    nc.tensor.matmul(
                        psv[:], xta[:, t // 4, k, ts(t % 4, P)], wv[:, k, :],
                        start=(k == 0), stop=(k == KO - 1),
                    )
                nc.vector.tensor_tensor(
                    v01[t][:, :, 0:64], psv[:], bvr[:], mybir.AluOpType.add
                )
                if t < 28:
                    # fp8 copy for the DoubleRow path (gpsimd is otherwise idle)
                    nc.gpsimd.tensor_copy(
                        v8[t // 2][:, t % 2, :, 0:64], v01[t][:, :, 0:64]
                    )

            def normalize(j):
                """Normalize chunk j's head outputs into ot[j]."""
                ppv0, ppv1 = ppv.pop(j)
                s0 = wp.tile([1, 512], BF16, name=f"s0_{j}", tag="s0", bufs=4)
                s1 = wp.tile([1, 512], BF16, name=f"s1_{j}", tag="s1", bufs=4)
                nc.scalar.copy(s0[:], ppv0[64:65, :])
                nc.vector.tensor_copy(s1[:], ppv1[64:65, :])
                pb = pp.tile([P, 512], F32, name=f"pb_{j}", tag="mx", bufs=2)
                nc.tensor.matmul(pb[0:64, :], ones64, s0[:], start=True, stop=True,
                                 tile_position=(0, 0))
                nc.tensor.matmul(pb[64:128, :], ones64, s1[:], start=True, stop=True,
                                 tile_position=(0, 64))
                rc = wp.tile([P, 512], F32, name=f"rc_{j}", tag="rc", bufs=4)
                nc.vector.reciprocal_approx_fast(rc[:], pb[:])
                nc.vector.tensor_mul(ot[j][0:64, :], ppv0[0:64, :], rc[0:64, :])
                nc.vector.tensor_mul(ot[j][64:128, :], ppv1[0:64, :], rc[64:128, :])

            def outproj(j, t):
                ob = wp.tile([P, D], BF16, name=f"ob_{t}", tag="ob", bufs=8)
                for n in range(2):
                    po = pp.tile([P, 512], F32, name=f"po_{t}_{n}", tag="mx", bufs=2)
                    nc.tensor.matmul(
                        po[:], ot[j][:, ts(t - 4 * j, P)], wo[:, ts(n, 512)],
                        start=True, stop=True,
                    )
                    # each half DMAs right after its own copy, on its own queue
                    if n == 1:
                        nc.scalar.copy(ob[:, ts(n, 512)], po[:])
                        nc.scalar.dma_start(out_d[ts(t, P), ts(n, 512)], ob[:, ts(n, 512)])
                    else:
                        nc.vector.tensor_copy(ob[:, ts(n, 512)], po[:])
                        nc.sync.dma_start(out_d[ts(t, P), ts(n, 512)], ob[:, ts(n, 512)])

            jf = NJ - 1

            def final_slice(sl):
                """Normalize + out-project one 128-token slice of the last
                chunk (its ppv columns are final after iteration 4*jf+sl)."""
                fpv0, fpv1 = ppv[jf]
                c = ts(sl, P)
                s0 = wp.tile([1, P], BF16, name=f"s0f_{sl}", tag="s0f", bufs=4)
                s1 = wp.tile([1, P], BF16, name=f"s1f_{sl}", tag="s1f", bufs=4)
                nc.scalar.copy(s0[:], fpv0[64:65, c])
                nc.vector.tensor_copy(s1[:], fpv1[64:65, c])
                pbf = pp.tile([P, P], F32, name=f"pbf_{sl}", tag="mx", bufs=2)
                nc.tensor.matmul(pbf[0:64, :], ones64, s0[:], start=True, stop=True,
                                 tile_position=(0, 0))
                nc.tensor.matmul(pbf[64:128, :], ones64, s1[:], start=True, stop=True,
                                 tile_position=(0, 64))
                rcf = wp.tile([P, P], F32, name=f"rcf_{sl}", tag="rcf", bufs=4)
                nc.vector.reciprocal_approx_fast(rcf[:], pbf[:])
                nc.vector.tensor_mul(ot[jf][0:64, c], fpv0[0:64, c], rcf[0:64, :])
                nc.vector.tensor_mul(ot[jf][64:128, c], fpv1[0:64, c], rcf[64:128, :])
                outproj(jf, 4 * jf + sl)

            # projections for chunk 0 up front
            proj_qk(0, "q")
            proj_qk(0, "k")
            pbv = pp.tile([P, 2, 64], F32, name="pbv", tag="mx", bufs=2)
            nc.tensor.matmul(pbv[:], ones1, bv, start=True, stop=True)
            nc.vector.tensor_copy(bvr[:], pbv[:])
            for i in range(NK):
                nc.gpsimd.memset(v01[i][:, :, 64:65], 1.0)
            for p in range(14):
                nc.gpsimd.memset(v8[p][:, :, :, 64:65], 1.0)

            nonstate = {"nexp": 0}
            for g in range(NJ):
                j = g
                nkj = 4 * (j + 1)
                ppv0 = pp.tile([65, 512], F32, name=f"ppv0_{j}", tag="ppv0", bufs=1)
                ppv1 = pp.tile([65, 512], F32, name=f"ppv1_{j}", tag="ppv1", bufs=1)
                ppv[j] = (ppv0, ppv1)

                # work units spread across this i-loop: projections for chunk
                # g+1 and the out-projection of the already-normalized chunk g-1
                units = []
                if g + 1 < NJ:
                    units.append(lambda g=g: proj_qk(g + 1, "q"))
                if g >= 1:
                    # this chunk's own V tiles: consumed at i >= 4g (the
                    # diagonal); schedule them early so the diag ppv LDWEIGHTS
                    # never waits on the DVE bias-add that produces v01
                    for t in range(4 * g, 4 * g + 2):
                        units.append(lambda t=t: proj_v(t))
                if g + 1 < NJ:
                    units.append(lambda g=g: proj_qk(g + 1, "k"))
                if g >= 1:
                    for t in range(4 * g + 2, 4 * g + 4):
                        units.append(lambda t=t: proj_v(t))
                # out-projection tiles spread in proportion to chunk length so
                # the 2-buffer mx psum pool never has 3 users in flight: chunk
                # g handles OP_SHARE[g] tiles in order (tile t of chunk t//4
                # becomes eligible 2 chunks later; chunk 6's tiles must go in
                # chunk 7 right after normalize(6)).
                for t in range(OP_START[g], OP_START[g] + OP_SHARE[g]):
                    units.append(lambda t=t: outproj(t // 4, t))
                nu = len(units)
                slots = {}
                for u in range(nu):
                    slots.setdefault(min(nkj - 1, 1 + (u * nkj) // (nu + 1)), []).append(units[u])

                # software-pipelined with lag 2+: exp gets >= two block-walls
                # before the in-order PE queue consumes it. Non-diag blocks go
                # through fp8 e + fp8 [V|1] DoubleRow matmuls (two k-tiles per
                # MM); diagonal blocks keep the masked bf16 Schraudolph path.
                # Scores psum bufs=2 suffices: exp(i) completes before
                # scores(i+2) needs its buffer.
                et = {}
                e8s = {}
                # diag blocks consume at lag dlag (3 normally, 1 on the last
                # chunk: the PE has nothing else left, so chase the epilogue);
                # fp8 pairs at ii = 2p+4 so they never wait on a FIFO'd exp.
                # PSUM accumulation is additive, so the j=NJ-1 interleaving of
                # diag blocks before the last pairs is safe (start was pair 0).
                dlag = 1 if j == NJ - 1 else 3
                for ii in range(nkj + dlag):
                    if ii < nkj:
                        i = ii
                        ps = pp.tile([P, 2, 512], F32, name=f"ps_{j}_{i}", tag="s", bufs=2)
                        m = i - 4 * j
                        c0 = 128 * m if m >= 1 else 0
                        nc.tensor.matmul(
                            ps[:, 0, c0:512], kt[i // 4][0:64, ts(i % 4, P)],
                            qt[j][0:64, c0:512], start=True, stop=True,
                        )
                        nc.tensor.matmul(
                            ps[:, 1, c0:512], kt[i // 4][64:128, ts(i % 4, P)],
                            qt[j][64:128, c0:512], start=True, stop=True,
                        )
                        if m >= 0:
                            e = wp.tile([P, 2, 512], BF16, name=f"e_{j}_{i}", tag="e", bufs=6)
                            et[i] = (e, c0)
                            # Schraudolph exp with fused causal mask on DVE
                            nc.vector.scalar_tensor_tensor(
                                e[:, :, c0:512].bitcast(I16), ps[:, :, c0:512],
                                SCA, maskt[:, m, :, c0:512], op0=MUL, op1=ADD,
                            )
                        else:
                            p8 = i // 2
                            if i % 2 == 0:
                                e8s[p8] = wp.tile(
                                    [P, 2, 2, 512], F8E4,
                                    name=f"e8_{j}_{p8}", tag="e8", bufs=4,
                                )
                            e8t = e8s[p8]
                            self_i = nonstate["nexp"]
                            nonstate["nexp"] += 1
                            if self_i % EXPMOD < NACT:
                                nc.scalar.activation(
                                    e8t[:, i % 2, :, :], ps[:], EXP, scale=0.125
                                )
                            else:
                                nc.vector.tensor_scalar(
                                    e8t[:, i % 2, :, :].bitcast(I8), ps[:],
                                    SCA8, SCB8, op0=MUL, op1=ADD,
                                )
                        if j == 0:
                            proj_v(i)
                        if i == 0 and j > 0:
                            normalize(j - 1)
                    if ii >= 4 and ii % 2 == 0 and (ii - 4) // 2 < 2 * j:
                        # DoubleRow pair (2p, 2p+1): fp8 e x fp8 [V|1]
                        p8 = (ii - 4) // 2
                        e8t = e8s.pop(p8)
                        nc.tensor.matmul(
                            ppv0[:], v8[p8][:, :, 0, 0:65], e8t[:, :, 0, :],
                            perf_mode=DR, start=(p8 == 0), stop=False,
                        )
                        nc.tensor.matmul(
                            ppv1[:], v8[p8][:, :, 1, 0:65], e8t[:, :, 1, :],
                            perf_mode=DR, start=(p8 == 0), stop=False,
                        )
                    if ii >= dlag and ii - dlag >= 4 * j:
                        i = ii - dlag
                        e, c0 = et.pop(i)
                        nc.tensor.matmul(
                            ppv0[:, c0:512], v01[i][:, 0, :], e[:, 0, c0:512],
                            start=(i == 0 and j == 0), stop=(i == nkj - 1),
                        )
                        nc.tensor.matmul(
                            ppv1[:, c0:512], v01[i][:, 1, :], e[:, 1, c0:512],
                            start=(i == 0 and j == 0), stop=(i == nkj - 1),
                        )
                        # last chunk: with column restriction, ppv columns
                        # [128*sl, 128*sl+128) are final once iteration
                        # i = 4*j + sl has accumulated -> stream the epilogue
                        # slice-by-slice instead of serializing it at the end
                        if j == NJ - 1 and 0 <= i - 4 * j < 4:
                            final_slice(i - 4 * j)
                    for fn in slots.get(ii, []):
                        fn()

    nc.compile()
    return nc


def _pack_w(w_slice, dt=ml_dtypes.bfloat16):
    """[HDC, D] weight slice -> transposed, chunked [P, KO, HDC]."""
    wt = np.ascontiguousarray(w_slice.T)          # [D, HDC]
    return np.ascontiguousarray(
        wt.reshape(KO, P, HDC).transpose(1, 0, 2)
    ).astype(dt)


def _make_in_maps(x, W_qkv, b_qkv, W_out, b_out):
    bf = ml_dtypes.bfloat16
    x = np.asarray(x, np.float32)
    W_qkv = np.asarray(W_qkv, np.float32)
    b_qkv = np.asarray(b_qkv, np.float32)
    W_out = np.asarray(W_out, np.float32)
    xt = np.ascontiguousarray(
        x.T.reshape(KO, P, NJ, 512).transpose(1, 2, 0, 3)
    ).astype(bf)
    in_maps = []
    for c in range(NCORES):
        r = slice(HDC * c, HDC * (c + 1))
        in_maps.append({
            "xt": xt,
            "wq": _pack_w(W_qkv[0 * D:1 * D][r]),
            "wk": _pack_w(W_qkv[1 * D:2 * D][r]),
            "wv": _pack_w(W_qkv[2 * D:3 * D][r]),
            "wo": np.ascontiguousarray(W_out[:, r].T).astype(bf),
            "bqk": np.ascontiguousarray(
                np.stack(
                    [b_qkv[0 * D:1 * D][r], b_qkv[1 * D:2 * D][r]], axis=1
                )
            ).astype(np.float32),
            "cst": np.ascontiguousarray(
                np.concatenate(
                    [b_qkv[2 * D:3 * D][r], np.ones(192, np.float32)]
                )[None, :]
            ).astype(bf),
        })
    return in_maps


_NC_CACHE = {}


def kernel(x, W_qkv, b_qkv, W_out, b_out):
    if "nc" not in _NC_CACHE:
        _NC_CACHE["nc"] = _build()
    nc = _NC_CACHE["nc"]
    in_maps = _make_in_maps(x, W_qkv, b_qkv, W_out, b_out)
    res = run_bass_kernel_spmd(nc, in_maps, core_ids=list(range(NCORES)))
    out = np.zeros((L, D), np.float32)
    for c in range(NCORES):
        out += res.results[c]["out"].astype(np.float32)
    out += np.asarray(b_out, np.float32)[None, :]
    return out

